# revision 33
# baseline (speedup 1.0000x reference)
"""MACE-style GNN message passing on 8 Trainium2 NeuronCores.

Only the l=0 (scalar) channel of the reference reaches the output, so the
network collapses algebraically: per edge, the radial MLP's last hidden
t3 (64) is dotted with a per-(sender-species, receiver-species) vector
Gamma[s,z] = W4_0 @ (hu[s] * delta[z]), where hu = w_embed@w_up and
delta[z] folds w_lin[0], w_sym[0], w_lin2[0] and w_readout.  Node energy
is then ae[z]+beta[z] + (1/16) * scatter_sum(eps_e).

Device pipeline (bf16 matmuls, fp32 geometry), software-pipelined with a
4-deep stage skew so every in-order engine queue always has ready work:
  - geometry (r via bit-trick rsqrt, envelope, bessel args) split in two
    chunks: chunk A on DVE, chunk B on the otherwise-idle GPSIMD
  - per 1024-edge block: 4 pair transposes of ef (PE) -> one [16,512]
    rhs, single L1 matmul, L2, then L3 is computed directly transposed
    (lhsT = t2 block, rhs = W3 half-matrices) so no separate t3
    transposes are needed; silu1+silu2 fused in one ACT call over a
    shared PSUM arena; product with gathered Gamma rows (DVE);
    per-subtile scatter matmul (one-hot stationary) accumulates
    msg[128 nodes, 64] per node tile in PSUM
  - epilogue: reduce h on DVE, scale + per-species constant, DMA out

Sharding: receivers range-partitioned (1000 nodes/core); per (core,
node-tile) edge groups padded to a uniform SEG subtiles of 128 so all
cores run one SPMD program.  Edges with r >= r_max are dropped on host
(so the polynomial-envelope cutoff mask is not needed on device; padding
slots are masked by zero one-hot rows and zero Gamma rows).
"""

import sys
import numpy as np

sys.path.insert(0, "/opt/trn_rl_repo")

import ml_dtypes

BF16 = ml_dtypes.bfloat16

R_MAX = 5.0
EPS = 1e-9
AVG = 16.0
N_NODES = 8000
Z = 10
K = 128
NB = 8
NCORES = 8
NPC = N_NODES // NCORES       # nodes per core (1000)
NT = 8                        # node tiles per core (128 nodes each)
SIN_DIRECT = False            # skip range reduction before ACT Sin

TRACE = False
LAST_RESULTS = None

_prog_cache = {}


def _build_program(SEG):
    """SPMD Bass program; SEG = 128-edge subtiles per 128-node tile."""
    from concourse import bass, bacc, mybir
    from concourse.tile import TileContext
    from contextlib import ExitStack

    f32 = mybir.dt.float32
    bf16 = mybir.dt.bfloat16
    i32 = mybir.dt.int32
    AF = mybir.ActivationFunctionType
    OP = mybir.AluOpType
    PSUM = bass.MemorySpace.PSUM

    S = NT * SEG              # total subtiles per core
    NBLK = S // 8             # 1024-edge blocks
    S3 = 3 * S

    nc = bacc.Bacc(None, target_bir_lowering=False)

    ve_d = nc.dram_tensor("ve", [128, S3], f32, kind="ExternalInput")
    g_d = nc.dram_tensor("gtab", [NBLK, 128, 512], bf16, kind="ExternalInput")
    ohr_d = nc.dram_tensor("ohr", [NBLK, 128, 1024], bf16, kind="ExternalInput")
    cf_d = nc.dram_tensor("constf", [128, 26], f32, kind="ExternalInput")
    cb_d = nc.dram_tensor("constb", [128, 512], bf16, kind="ExternalInput")
    out_d = nc.dram_tensor("out", [128, 8], f32, kind="ExternalOutput")

    # geometry chunk split (in blocks): chunk A on DVE, chunk B on GPSIMD.
    # GPSIMD is slow and serial, so it only covers the tail blocks it can
    # finish before the pipeline reaches them.
    BLK_A = max(1, NBLK - 4)

    with TileContext(nc) as tc:
        with ExitStack() as stack:
            cp = stack.enter_context(tc.tile_pool(name="const", bufs=1))
            geo = stack.enter_context(tc.tile_pool(name="geo", bufs=1))
            efsp = stack.enter_context(tc.tile_pool(name="efsp", bufs=3))
            gp = stack.enter_context(tc.tile_pool(name="gp", bufs=5))
            ohp = stack.enter_context(tc.tile_pool(name="ohp", bufs=5))
            ttp = stack.enter_context(tc.tile_pool(name="ttp", bufs=3))
            t3p = stack.enter_context(tc.tile_pool(name="t3p", bufs=3))
            qp = stack.enter_context(tc.tile_pool(name="qp", bufs=3))
            outp = stack.enter_context(tc.tile_pool(name="outp", bufs=1))
            pefp = stack.enter_context(tc.tile_pool(name="pefp", bufs=2, space=PSUM))
            par = stack.enter_context(tc.tile_pool(name="par", bufs=2, space=PSUM))
            pq3 = stack.enter_context(tc.tile_pool(name="pq3", bufs=1, space=PSUM))
            pmsg = stack.enter_context(tc.tile_pool(name="pmsg", bufs=1, space=PSUM))

            # ---- constants (issue small/critical DMAs first) ----
            CTF = cp.tile([128, 26], f32)
            nc.sync.dma_start(CTF[:], cf_d[:])
            VE = geo.tile([128, S3], f32)
            CUT = 24 * max(1, NBLK - 4)
            nc.sync.dma_start(VE[:, 0:CUT], ve_d[:, 0:CUT])
            nc.sync.dma_start(VE[:, CUT:S3], ve_d[:, CUT:S3])
            CTB = cp.tile([128, 512], bf16)
            nc.sync.dma_start(CTB[:], cb_d[:])
            CB8 = CTF[:, 0:8]
            CNODE = CTF[:, 8:16]
            ONEI = CTF[:, 16:17].bitcast(i32)
            MAGIC = CTF[:, 17:18].bitcast(i32)
            # scalar constants as broadcastable columns (Pool engine has no
            # dual-scalar TensorScalar, so chunk B uses TT with these)
            CCOL = {v: CTF[:, 18 + k:19 + k] for k, v in enumerate(
                [EPS, 0.5, 1.5, 15.0, 35.0, 21.0, 1.0, 1.0 / R_MAX])}
            W1PK = CTB[0:16, 0:128]
            W2BD = CTB[:, 128:256]
            W3XY = CTB[:, 256:384]
            I128 = CTB[:, 384:512]

            tc.strict_bb_all_engine_barrier()

            def emit_geo(E, b0, b1, Ebit=None):
                """Emit geometry for blocks [b0, b1) on engine E; returns
                the EFB tile plus the deferred sin/efb thunks.  Ebit runs
                the two integer bit-trick ops (Pool can't shift int32)."""
                if Ebit is None:
                    Ebit = E
                nb = b1 - b0
                sb = 8 * nb
                w8 = 64 * nb
                VEc = VE[:, 24 * b0:24 * b1]
                SQ = geo.tile([128, 3 * sb], f32, name="SQ")
                SC = geo.tile([128, 8 * sb], f32, name="SC")
                TH = geo.tile([128, w8], f32, name="TH")
                SH = geo.tile([128, w8], f32, name="SH")
                EFB = geo.tile([128, w8], bf16, name="EFB")

                def sl(i):
                    return SC[:, i * sb:(i + 1) * sb]

                ss, y, h2, q2, r_, x, u1, u2 = (sl(i) for i in range(8))
                y_i = y.bitcast(i32)
                dve = E is nc.vector

                def cc(v, w=sb):
                    return CCOL[v].broadcast_to([128, w])

                E.tensor_tensor(SQ[:], VEc, VEc, OP.mult)
                SQ3 = SQ[:].rearrange("p (s c) -> p c s", c=3)
                E.tensor_tensor(ss, SQ3[:, 0], SQ3[:, 1], OP.add)
                E.tensor_tensor(ss, ss, SQ3[:, 2], OP.add)
                E.tensor_tensor(ss, ss, cc(EPS), OP.add)
                Ebit.tensor_tensor(y_i, ss.bitcast(i32),
                                   ONEI.broadcast_to([128, sb]),
                                   OP.arith_shift_right)
                Ebit.tensor_tensor(y_i, MAGIC.broadcast_to([128, sb]), y_i,
                                   OP.subtract)
                E.tensor_tensor(h2, ss, cc(0.5), OP.mult)
                for _ in range(2):
                    E.tensor_tensor(q2, y, y, OP.mult)
                    E.tensor_tensor(q2, q2, h2, OP.mult)
                    if dve:
                        E.tensor_scalar(q2, q2, -1.0, 1.5, OP.mult, OP.add)
                    else:
                        E.tensor_tensor(q2, cc(1.5), q2, OP.subtract)
                    E.tensor_tensor(y, y, q2, OP.mult)
                E.tensor_tensor(r_, ss, y, OP.mult)      # r = sqrt(ss)
                # envelope (no cutoff mask: r<R_MAX guaranteed for real
                # edges; pad slots are masked downstream)
                E.tensor_tensor(x, r_, cc(1.0 / R_MAX), OP.mult)
                E.tensor_tensor(u1, x, x, OP.mult)
                E.tensor_tensor(u1, u1, u1, OP.mult)
                E.tensor_tensor(u1, u1, x, OP.mult)      # x^5
                if dve:
                    E.tensor_scalar(u2, x, -15.0, 35.0, OP.mult, OP.add)
                else:
                    E.tensor_tensor(u2, x, cc(15.0), OP.mult)
                    E.tensor_tensor(u2, cc(35.0), u2, OP.subtract)
                E.tensor_tensor(u2, u2, x, OP.mult)
                if dve:
                    E.scalar_tensor_tensor(u1, u2, -21.0, u1, OP.add, OP.mult)
                    # w = (env+1) * rinv ; sqrt(2/R) folded into W1
                    E.scalar_tensor_tensor(u2, u1, 1.0, y, OP.add, OP.mult)
                else:
                    E.tensor_tensor(u2, u2, cc(21.0), OP.subtract)
                    E.tensor_tensor(u1, u2, u1, OP.mult)
                    E.tensor_tensor(u1, u1, cc(1.0), OP.add)
                    E.tensor_tensor(u2, u1, y, OP.mult)
                E.tensor_tensor(
                    TH[:].rearrange("p (s b) -> p s b", b=8),
                    CB8.unsqueeze(1).broadcast_to([128, sb, 8]),
                    r_.unsqueeze(2).broadcast_to([128, sb, 8]), OP.mult)
                if SIN_DIRECT:
                    SAf = TH
                else:
                    KI = geo.tile([128, w8], i32, name="KI")
                    KF = geo.tile([128, w8], f32, name="KF")
                    SA = geo.tile([128, w8], f32, name="SA")
                    GTt = geo.tile([128, w8], f32, name="GTt")
                    if dve:
                        E.tensor_copy(KI[:], TH[:])
                        E.tensor_copy(KF[:], KI[:])
                        E.tensor_tensor(SA[:], TH[:], KF[:], OP.subtract)
                        E.tensor_scalar(GTt[:], SA[:], 0.5, None, OP.is_gt)
                        E.scalar_tensor_tensor(
                            SA[:], GTt[:], -1.0, SA[:], OP.mult, OP.add)
                    else:
                        # comparison-free fold, correct for either cast
                        # rounding mode: sa = th-castrt(th+0.5), then
                        # sa -= castrt(sa)
                        E.tensor_tensor(GTt[:], TH[:], cc(0.5, w8), OP.add)
                        E.tensor_copy(KI[:], GTt[:])
                        E.tensor_copy(KF[:], KI[:])
                        E.tensor_tensor(SA[:], TH[:], KF[:], OP.subtract)
                        E.tensor_copy(KI[:], SA[:])
                        E.tensor_copy(KF[:], KI[:])
                        E.tensor_tensor(SA[:], SA[:], KF[:], OP.subtract)
                    SAf = SA

                def sin_op():
                    nc.scalar.activation(SH[:], SAf[:], AF.Sin,
                                         scale=float(2 * np.pi))

                def efb_op():
                    E.tensor_tensor(
                        EFB[:].rearrange("p (s b) -> p s b", b=8),
                        SH[:].rearrange("p (s b) -> p s b", b=8),
                        u2.unsqueeze(2).broadcast_to([128, sb, 8]), OP.mult)

                return {"EFB": EFB, "sin": sin_op, "efb": efb_op}

            gA = emit_geo(nc.vector, 0, BLK_A)
            gB = (emit_geo(nc.gpsimd, BLK_A, NBLK, Ebit=nc.vector)
                  if BLK_A < NBLK else None)
            gA["sin"]()
            gA["efb"]()
            if gB is not None:
                gB["sin"]()
                gB["efb"]()

            # ---- software-pipelined block loop ----
            # iteration i issues: efT+L1(i-1 inputs)... stage skew:
            #   efT(i) -> L1(i-1) -> [arena silu: t1(i-1)|t2(i-2)]
            #   L2(i-2) -> L3T(i-3)+silu3(i-3)+product(i-3) -> scatter(i-4)
            MSG = pmsg.tile([128, 512], f32, tag="msg")
            efs = {}
            tts = {}   # TT12 arena tiles: [0:512]=t1(i-1), [512:1024]=t2(i-2)
            t3s = {}
            qss = {}
            gts = {}
            ohrs = {}
            NITER = NBLK + 4
            for i in range(NITER):
                if i < NBLK:
                    gts[i] = gp.tile([128, 512], bf16, tag="gt", name="gt")
                    nc.sync.dma_start(gts[i][:], g_d[i])
                    ohrs[i] = ohp.tile([128, 1024], bf16, tag="ohr",
                                       name="ohrt")
                    nc.sync.dma_start(ohrs[i][:], ohr_d[i])

                # stage 1: 4 pair transposes of ef (PE) + copy (DVE)
                if i < NBLK:
                    if i < BLK_A:
                        EFBc, off = gA["EFB"], i
                    else:
                        EFBc, off = gB["EFB"], i - BLK_A
                    pef = pefp.tile([16, 512], bf16, tag="pef")
                    for p in range(4):
                        nc.tensor.transpose(
                            pef[:, 128 * p:128 * p + 128],
                            EFBc[:, 64 * off + 16 * p:64 * off + 16 * p + 16],
                            I128)
                    efs[i] = efsp.tile([16, 512], bf16, tag="efs", name="efs")
                    nc.vector.tensor_copy(efs[i][:], pef[:])

                # stages 2+3: L1(i-1) + L2(i-2) into shared arena,
                # then one fused silu over both
                j1, j2 = i - 1, i - 2
                if 0 <= j1 < NBLK or 0 <= j2 < NBLK:
                    arena = par.tile([128, 1024], f32, tag="arena")
                    lo = 0 if 0 <= j1 < NBLK else 512
                    hi = 1024 if 0 <= j2 < NBLK else 512
                    if 0 <= j1 < NBLK:
                        nc.tensor.matmul(arena[:, 0:512], W1PK, efs[j1][:],
                                         start=True, stop=True)
                        del efs[j1]
                    if 0 <= j2 < NBLK:
                        nc.tensor.matmul(arena[:, 512:1024], W2BD,
                                         tts[j2][:, 0:512],
                                         start=True, stop=True)
                    tt = ttp.tile([128, 1024], bf16, tag="tt", name="tt")
                    nc.scalar.activation(tt[:, lo:hi], arena[:, lo:hi],
                                         AF.Silu)
                    if 0 <= j1 < NBLK:
                        tts[j1] = tt
                    if 0 <= j2 < NBLK:
                        # move finished t2 view into its own dict slot
                        t3s[j2] = ("t2", tt)

                # stage 4: L3 computed transposed (PE) + silu3 (ACT)
                #          + product with Gamma (DVE)
                j = i - 3
                if 0 <= j < NBLK:
                    _, ttj = t3s.pop(j)
                    q3 = pq3.tile([128, 512], f32, tag="q3")
                    for c in range(4):
                        t2blk = ttj[:, 512 + 128 * c:512 + 128 * c + 128]
                        # rhs = [W3X | W3Y] packed contiguously in CTB
                        nc.tensor.matmul(
                            q3[:, 128 * c:128 * c + 128], t2blk, W3XY,
                            start=True, stop=True)
                    t3e = t3p.tile([128, 512], bf16, tag="t3e", name="t3e")
                    nc.scalar.activation(t3e[:], q3[:], AF.Silu)
                    qss[j] = qp.tile([128, 512], bf16, tag="qs", name="qs")
                    nc.vector.tensor_tensor(qss[j][:], t3e[:], gts[j][:],
                                            OP.mult)
                    del gts[j]
                    if j - 1 in tts:
                        del tts[j - 1]

                # stage 5: scatter (PE)
                j = i - 4
                if 0 <= j < NBLK:
                    for k in range(8):
                        s = 8 * j + k
                        nt_ = s // SEG
                        qcol = 128 * (k // 2) + 64 * (k % 2)
                        nc.tensor.matmul(
                            MSG[:, 64 * nt_:64 * nt_ + 64],
                            ohrs[j][:, 128 * k:128 * k + 128],
                            qss[j][:, qcol:qcol + 64],
                            start=(s % SEG == 0), stop=(s % SEG == SEG - 1),
                            skip_group_check=True)
                    del qss[j], ohrs[j]

            # ---- epilogue ----
            MSUM = outp.tile([128, 8], f32)
            nc.vector.tensor_reduce(
                MSUM[:], MSG[:].rearrange("p (n h) -> p n h", h=64),
                mybir.AxisListType.X, OP.add)
            OUTT = outp.tile([128, 8], f32)
            nc.vector.scalar_tensor_tensor(
                OUTT[:], MSUM[:], 1.0 / AVG, CNODE, OP.mult, OP.add)
            nc.sync.dma_start(out_d[:], OUTT[:])

    nc.compile()
    return nc


def _host_prep(inputs):
    pos = np.asarray(inputs["positions"], np.float32)
    shifts = np.asarray(inputs["shifts"], np.float32)
    ei = np.asarray(inputs["edge_index"])
    species = np.asarray(inputs["species"]).astype(np.int64)
    ae = np.asarray(inputs["atomic_energies"], np.float32)
    w_embed = np.asarray(inputs["w_embed"], np.float32)
    w_up = np.asarray(inputs["w_up"], np.float32)
    W1 = np.asarray(inputs["W1"], np.float32)
    W2 = np.asarray(inputs["W2"], np.float32)
    W3 = np.asarray(inputs["W3"], np.float32)
    W4 = np.asarray(inputs["W4"], np.float32)
    w_lin = np.asarray(inputs["w_lin"], np.float32)
    w_skip = np.asarray(inputs["w_skip"], np.float32)
    w_sym = np.asarray(inputs["w_sym"], np.float32)
    w_lin2 = np.asarray(inputs["w_lin2"], np.float32)
    w_ro = np.asarray(inputs["w_readout"], np.float32)

    # collapsed weight tables
    hu = w_embed @ w_up                                   # [Z,K]
    alpha = w_lin2[0] @ w_ro                              # [K]
    delta = np.einsum("qk,zk,k->zq", w_lin[0], w_sym[0], alpha)  # [Z,K]
    W4_0 = np.ascontiguousarray(W4.reshape(64, K, 4)[:, :, 0])   # [64,K]
    Gamma = np.einsum("hk,sk,zk->szh", W4_0, hu, delta)   # [Z,Z,64]
    sct = np.einsum("zk,zkj->zj", w_embed, w_skip) / np.sqrt(Z)
    cz = ae + sct @ w_ro                                  # [Z]

    send, recv = ei[0].astype(np.int64), ei[1].astype(np.int64)
    vec = pos[recv] - pos[send] + shifts
    rsq = (vec * vec).sum(-1)
    keep = rsq < (R_MAX * R_MAX + 1e-3)
    vec = vec[keep]
    sp_s = species[send[keep]]
    recv = recv[keep]
    sp_r = species[recv]

    core = recv // NPC
    loc = recv % NPC
    ntile = loc // 128
    lrow = loc % 128

    order = np.lexsort((ntile, core))
    vec, sp_s, sp_r, lrow = vec[order], sp_s[order], sp_r[order], lrow[order]
    core, ntile = core[order], ntile[order]
    gid = core * NT + ntile
    counts = np.bincount(gid, minlength=NCORES * NT)
    SEG = int(np.ceil(counts.max() / 128))
    S = NT * SEG
    NBLK = S // 8

    VEa = np.zeros((NCORES, 128, S, 3), np.float32)
    VEa[:, :, :, 0] = 2.0   # pad edges: r=2, harmless (masked downstream)
    Ga = np.zeros((NCORES, NBLK, 128, 8, 64), BF16)
    OHa = np.zeros((NCORES, NBLK, 128, 8, 128), BF16)

    Gedge = Gamma[sp_s, sp_r].astype(BF16)   # [E, 64]

    starts = np.zeros(NCORES * NT + 1, np.int64)
    np.cumsum(counts, out=starts[1:])
    for c_ in range(NCORES):
        for t in range(NT):
            g = c_ * NT + t
            a, b = starts[g], starts[g + 1]
            n = b - a
            idx = np.arange(n)
            sub = t * SEG + idx // 128        # global subtile
            row = idx % 128
            blk = sub // 8
            jj = sub % 8
            VEa[c_, row, sub, :] = vec[a:b]
            Ga[c_, blk, row, jj, :] = Gedge[a:b]
            OHa[c_, blk, row, jj, lrow[a:b]] = 1.0

    cnode = np.zeros((NCORES, 128, 8), np.float32)
    for c_ in range(NCORES):
        spc = species[c_ * NPC:(c_ + 1) * NPC]
        czc = cz[spc]
        for t in range(NT):
            nloc = min(128, NPC - t * 128)
            cnode[c_, :nloc, t] = czc[t * 128:t * 128 + nloc]

    n_ = np.arange(1, NB + 1, dtype=np.float32)
    cb8 = n_ / (2.0 * R_MAX)    # th = cb*r ; sin(2*pi*th) = sin(n*pi*r/R)
    constf = np.zeros((NCORES, 128, 26), np.float32)
    constf[:, :, 0:8] = cb8[None, None, :]
    constf[:, :, 8:16] = cnode
    constf[:, :, 16] = np.full((1,), 1, np.int32).view(np.float32)[0]
    constf[:, :, 17] = np.full((1,), 0x5F3759DF, np.int32).view(np.float32)[0]
    for k, v in enumerate([EPS, 0.5, 1.5, 15.0, 35.0, 21.0, 1.0, 1.0 / R_MAX]):
        constf[:, :, 18 + k] = v

    # bf16 weight consts (same for all cores); sqrt(2/R) folded into W1
    W1s = W1 * np.sqrt(2.0 / R_MAX)
    cb = np.zeros((128, 512), np.float32)
    cb[0:8, 0:64] = W1s          # even subtile of pair -> out 0:64
    cb[8:16, 64:128] = W1s       # odd subtile -> out 64:128
    cb[0:64, 128:192] = W2
    cb[64:128, 192:256] = W2
    cb[0:64, 256:320] = W3       # W3X: top half
    cb[64:128, 320:384] = W3     # W3Y: bottom half
    cb[:, 384:512] = np.eye(128, dtype=np.float32)
    constb = cb.astype(BF16)

    return SEG, VEa, Ga, OHa, constf, constb


def kernel(**inputs):
    global LAST_RESULTS
    from concourse.bass_utils import run_bass_kernel_spmd

    SEG, VEa, Ga, OHa, constf, constb = _host_prep(inputs)
    S = NT * SEG
    NBLK = S // 8
    if SEG not in _prog_cache:
        _prog_cache[SEG] = _build_program(SEG)
    nc = _prog_cache[SEG]

    in_maps = []
    for c_ in range(NCORES):
        m = {
            "ve": np.ascontiguousarray(VEa[c_].reshape(128, 3 * S)),
            "gtab": np.ascontiguousarray(Ga[c_].reshape(NBLK, 128, 512)),
            "ohr": np.ascontiguousarray(OHa[c_].reshape(NBLK, 128, 1024)),
            "constf": np.ascontiguousarray(constf[c_]),
            "constb": constb,
        }
        in_maps.append(m)

    res = run_bass_kernel_spmd(
        nc, in_maps, core_ids=list(range(NCORES)), trace=TRACE)
    LAST_RESULTS = res

    out = np.concatenate(
        [res.results[c_]["out"].T.reshape(1024)[:NPC] for c_ in range(NCORES)])
    return out.astype(np.float32)
